# revision 16
# baseline (speedup 1.0000x reference)
"""Trainium2 Bass kernel for nn_Attention1 (squared-difference attention).

Math (per batch b):
    q  = t1 @ Wq,  k = t1 @ Wk,  p = t2 @ Wp,  v = t1 @ Wv     (per head h: 64 dims)
    raw   = q_h @ (k_h - p_h)^T                                 [n, n]
    logit = raw^2 / sqrt(512)
    attn  = softmax(logit, axis=-1)
    out   = concat_h(attn @ v_h) @ Wo + bo

Sharding: 16 (b, h) pairs over 8 cores -> 2 heads + 1 batch per core.
Each core computes its 2 heads end-to-end including its partial
out-projection (row-sharded Wo); host sums the 4 partials per batch
(the "all-reduce after to_out" of the hint, done at unshard time).

Device dataflow per core (all n=2048, d=64, 2 heads):
  x1t/x2t = t1[b]^T, t2[b]^T  (pre-transposed on host -> clean DMA loads)
  qT, kpT:  [128(2h*64d), 2048] = W^T @ x^T on PE  (kp uses -Wp folded on host)
  v:        [n, 128] natural layout, lhsT = x1t slices
  dots:     psum[i128, j] = qT_h^T-slice @ kpT_h   (K=64, heads row-packed)
  TTR(DVE): sqneg = -(raw*raw)/sqrt(512)  [sbuf], accum = min -> -rowmax
  exp(ACT): s = exp(-sqneg - rowmax) bf16, accum_out -> rowsum r
  DMA-transpose s [i,j] -> sT [j,i] (bf16, XBAR)
  attn@v:   psum[2h*64d, i512] += v_t^T-slice @ sT  (heads col-packed)
  evict * (1/r broadcast), outproj MM vs Wo slice, + bo, DMA out.
"""

import os
import sys

import numpy as np

for _p in ("/opt/trn_rl_repo",):
    if _p not in sys.path:
        sys.path.insert(0, _p)

import concourse.bass as bass
import concourse.mybir as mybir
import concourse.tile as tile
from concourse.bass_utils import run_bass_kernel_spmd
from concourse.vector_clock import ScopedClock, VectorClock


def _patched_drain_and_barrier(self, tick_clock, wait_clock):
    """Kernel-tail drain emitting one wait per proc.

    The stock tail puts every proc's sem wait on a single Drain; this
    container's walrus ("Too many sync wait commands") only accepts one
    wait per instruction, so split the clock per proc and use the NRT
    pseudo sync barrier in place of the direct EVSEM butterfly.
    """
    nc = self.nc
    gc = tick_clock.global_clock
    nprocs = len(gc)
    for i in range(nprocs):
        t = gc[i]
        if t <= 0:
            continue
        vc = VectorClock([0] * nprocs)
        vc.require_at_least(i, t)
        d = nc.sync.drain()
        wait_clock.add_sem_waits(d.ins, ScopedClock({None: vc}))
    nc.all_engine_barrier()
    popped = nc._tile_sem_poison_stack.pop()
    assert popped is self._sem_poison
    nc.clear_and_free_semaphores(list(self.sems.allocated().values()))
    nc.all_engine_barrier()


tile.TileContext._drain_and_barrier = _patched_drain_and_barrier


def _split_multi_waits(nc):
    """This container's walrus accepts one sync wait per instruction;
    move extra waits onto EventSemaphore instructions inserted before."""
    import bass_rust

    n = 0
    for fn in nc.m.functions:
        for bb in fn.blocks:
            il = bb.instructions
            out = []
            for inst in il:
                si = inst.sync_info
                waits = list(si.on_wait) if (si is not None and si.on_wait) else []
                if len(waits) > 1:
                    for w in waits[:-1]:
                        n += 1
                        ev = mybir.InstEventSemaphore(
                            name=f"SW-{n}-{inst.name}",
                            engine=inst.engine,
                            debug=inst.debug,
                            sync_info=bass_rust.SyncInfo(on_wait=[w], on_update=[]),
                        )
                        out.append(ev)
                    si.on_wait = [waits[-1]]
                out.append(inst)
            il[:] = out
    return n

FP32 = mybir.dt.float32
BF16 = mybir.dt.bfloat16
AF = mybir.ActivationFunctionType
ALU = mybir.AluOpType
FP32R = mybir.dt.float32r


def _r(ap):
    return ap.bitcast(FP32R)

DIM = 512
HEADS = 8
DH = 64
N = 2048
B = 2
SCALE = float(DIM) ** -0.5
SQRT_SCALE = float(DIM) ** -0.25
NCORES = 8

NI = N // 128          # 16 i-chunks of 128
NG = 4                 # i512 groups
CPG = NI // NG         # chunks per group = 4
NJ = N // 128          # 16 j-chunks
JW = 1024              # dots psum width (j)


def build_bass() -> bass.Bass:
    nc = bass.Bass()

    x1t = nc.dram_tensor("x1t", [DIM, N], FP32, kind="ExternalInput")
    x2t = nc.dram_tensor("x2t", [DIM, N], FP32, kind="ExternalInput")
    wq = nc.dram_tensor("wq", [DIM, 128], FP32, kind="ExternalInput")
    wk = nc.dram_tensor("wk", [DIM, 128], FP32, kind="ExternalInput")
    wpn = nc.dram_tensor("wpn", [DIM, 128], FP32, kind="ExternalInput")
    wv = nc.dram_tensor("wv", [DIM, 128], FP32, kind="ExternalInput")
    wo = nc.dram_tensor("wo", [128, DIM], FP32, kind="ExternalInput")
    bo_b = nc.dram_tensor("bo_b", [128, DIM], FP32, kind="ExternalInput")
    out = nc.dram_tensor("out", [N, DIM], FP32, kind="ExternalOutput")
    rscr = nc.dram_tensor("rscr", [2, N], FP32)

    with tile.TileContext(nc) as tc:
        with (
            tc.tile_pool(name="persist", bufs=1) as persist,
            tc.tile_pool(name="work", bufs=3) as work,
            tc.tile_pool(name="sq", bufs=3) as sqpool,
            tc.tile_pool(name="spool", bufs=4) as spool,
            tc.tile_pool(name="stp", bufs=1) as stp,
            tc.tile_pool(name="psd", bufs=3, space="PSUM") as psd_pool,
            tc.tile_pool(name="pss", bufs=2, space="PSUM") as pss_pool,
        ):
            # ---------------- load inputs ----------------
            x1 = []
            x2 = []
            for kc in range(4):
                t = persist.tile([128, N], FP32R, tag=f"x1_{kc}", name=f"x1_{kc}")
                nc.gpsimd.dma_start(t[:], _r(x1t[kc * 128:(kc + 1) * 128, :]))
                x1.append(t)
            for kc in range(4):
                t = persist.tile([128, N], FP32R, tag=f"x2_{kc}", name=f"x2_{kc}")
                nc.gpsimd.dma_start(t[:], _r(x2t[kc * 128:(kc + 1) * 128, :]))
                x2.append(t)

            def load_w(dram, name):
                t = persist.tile([128, 4, 128], FP32R, tag=name)
                nc.gpsimd.dma_start(
                    t[:], _r(dram[:].rearrange("(kc p) m -> p kc m", p=128))
                )
                return t

            wq_sb = load_w(wq, "wq")
            wk_sb = load_w(wk, "wk")
            wpn_sb = load_w(wpn, "wpn")
            wv_sb = load_w(wv, "wv")

            wo_sb = persist.tile([128, DIM], FP32R, tag="wo", name="wo")
            nc.gpsimd.dma_start(wo_sb[:], _r(wo[:]))
            bo_sb = persist.tile([128, DIM], FP32, tag="bo", name="bo")
            nc.gpsimd.dma_start(bo_sb[:], bo_b[:])

            # ---------------- projections ----------------
            qT = persist.tile([128, N], FP32R, tag="qT", name="qT")
            kpT = persist.tile([128, N], FP32R, tag="kpT", name="kpT")

            # qT = Wq^T @ x1t ; kpT = Wk^T @ x1t - Wp^T @ x2t
            for nb in range(4):
                psq = pss_pool.tile([128, 512], FP32, tag="pss", name="pss")
                for kc in range(4):
                    nc.tensor.matmul(
                        psq[:],
                        wq_sb[:, kc, :],
                        x1[kc][:, nb * 512:(nb + 1) * 512],
                        start=(kc == 0),
                        stop=(kc == 3),
                    )
                nc.vector.tensor_copy(qT[:, nb * 512:(nb + 1) * 512], psq[:])

            psk = [pss_pool.tile([128, 512], FP32, tag="pss", name="pss") for _ in range(4)]
            for kc in range(4):
                for nb in range(4):
                    nc.tensor.matmul(
                        psk[nb][:],
                        wk_sb[:, kc, :],
                        x1[kc][:, nb * 512:(nb + 1) * 512],
                        start=(kc == 0),
                        stop=False,
                    )
                for nb in range(4):
                    nc.tensor.matmul(
                        psk[nb][:],
                        _r(wpn_sb[:, kc, :]),
                        x2[kc][:, nb * 512:(nb + 1) * 512],
                        start=False,
                        stop=(kc == 3),
                    )
            for nb in range(4):
                nc.vector.tensor_copy(kpT[:, nb * 512:(nb + 1) * 512], psk[nb][:])

            # v natural [n, 128], bf16, per j-chunk tiles
            v_sb = []
            for t in range(NJ):
                vt = persist.tile([128, 128], BF16, tag=f"v_{t}", name=f"v_{t}")
                psv = pss_pool.tile([128, 512], FP32, tag="pss", name="pss")
                for kc in range(4):
                    nc.tensor.matmul(
                        psv[:, 0:128],
                        x1[kc][:, t * 128:(t + 1) * 128],
                        wv_sb[:, kc, :],
                        start=(kc == 0),
                        stop=(kc == 3),
                    )
                nc.vector.tensor_copy(vt[:], psv[:, 0:128])
                v_sb.append(vt)

            # r accumulators: per head, one column per i-chunk
            rcol = [persist.tile([128, NI], FP32, tag=f"rcol{h}", name=f"rcol{h}") for h in range(2)]

            # ---------------- attention ----------------
            pending = []
            st_map = {}

            def flush_one():
                sq_p, negm_p, h_p, gc_p, c_p, g_p = pending.pop(0)
                if g_p not in st_map:
                    st_map[g_p] = [
                        stp.tile([128, NJ * 512], BF16, tag=f"st{hh}",
                                 name=f"st{hh}")
                        for hh in range(2)
                    ]
                s_t = spool.tile([128, N], BF16, tag="s", name="s")
                nc.scalar.activation(
                    s_t[:], sq_p[:], AF.Exp, bias=negm_p[:], scale=1.0,
                    accum_out=rcol[h_p][:, gc_p:gc_p + 1],
                )
                nc.sync.dma_start_transpose(
                    st_map[g_p][h_p][:].rearrange("p (t i) -> p t i", i=512)[
                        :, :, c_p * 128:(c_p + 1) * 128
                    ],
                    s_t[:],
                )

            def emit_iter(g, c, h, flush):
                gc = g * CPG + c
                i0 = gc * 128
                hp = h * 64
                sq = sqpool.tile([128, N], FP32, tag="sq", name="sq")
                negm = work.tile([128, 1], FP32, tag="negm", name="negm")
                for jh in range(N // JW):
                    psd = psd_pool.tile([128, JW], FP32, tag="psd", name="psd")
                    for jq in range(JW // 512):
                        j0 = jh * JW + jq * 512
                        nc.tensor.matmul(
                            psd[:, jq * 512:(jq + 1) * 512],
                            qT[hp:hp + 64, i0:i0 + 128],
                            kpT[hp:hp + 64, j0:j0 + 512],
                            start=True,
                            stop=True,
                        )
                    nc.scalar.activation(
                        sq[:, jh * JW:(jh + 1) * JW], psd[:], AF.Square,
                        scale=SQRT_SCALE,
                    )
                nc.vector.tensor_reduce(
                    negm[:], sq[:], mybir.AxisListType.X, ALU.max, negate=True,
                )
                pending.append((sq, negm, h, gc, c, g))
                if flush and len(pending) > 1:
                    flush_one()

            for g in range(NG):
                for c in range(CPG):
                    for h in range(2):
                        if g > 0 and c == 0 and h == 0:
                            continue  # hoisted before av(g-1) below
                        emit_iter(g, c, h, True)
                if g + 1 < NG:
                    # hoist next group's first dots so PE has work while
                    # av(g) waits on this group's final transposes
                    emit_iter(g + 1, 0, 0, False)
                while pending and pending[0][5] == g:
                    flush_one()

                # 1/r broadcast round trip, before av to hide its latency
                rb = work.tile([128, 512], FP32, tag="rb", name="rb")
                for h in range(2):
                    rrec = work.tile([128, CPG], FP32, tag="rrec", name="rrec")
                    nc.vector.reciprocal(
                        rrec[:], rcol[h][:, g * CPG:(g + 1) * CPG]
                    )
                    nc.sync.dma_start(
                        rscr[h, g * 512:(g + 1) * 512].rearrange(
                            "(cc p) -> p cc", p=128
                        ),
                        rrec[:],
                    )
                    nc.sync.dma_start(
                        rb[h * 64:(h + 1) * 64, :],
                        rscr[h, g * 512:(g + 1) * 512]
                        .unsqueeze(0)
                        .broadcast_to((64, 512)),
                    )

                st_g = st_map[g]
                pso = pss_pool.tile([128, 512], FP32, tag="pss", name="pss")
                for t in range(NJ):
                    for h in range(2):
                        nc.tensor.matmul(
                            pso[h * 64:(h + 1) * 64, :],
                            v_sb[t][:, h * 64:(h + 1) * 64],
                            st_g[h][:, t * 512:(t + 1) * 512],
                            start=(t == 0),
                            stop=(t == NJ - 1),
                            tile_position=(0, h * 64),
                            skip_group_check=True,
                        )

                attn_sb = work.tile([128, 512], FP32R, tag="attn", name="attn")
                nc.vector.tensor_mul(attn_sb[:], pso[:], rb[:])

                for c in range(CPG):
                    psp = pss_pool.tile([128, 512], FP32, tag="pss", name="pss")
                    nc.tensor.matmul(
                        psp[:],
                        attn_sb[:, c * 128:(c + 1) * 128],
                        wo_sb[:],
                        start=True,
                        stop=True,
                    )
                    out_sb = work.tile([128, 512], FP32, tag="out", name="out")
                    nc.vector.tensor_add(out_sb[:], psp[:], bo_sb[:])
                    nc.gpsimd.dma_start(
                        out[(g * CPG + c) * 128:(g * CPG + c + 1) * 128, :],
                        out_sb[:],
                    )

    _split_multi_waits(nc)
    return nc


_NC = None


def _get_nc():
    global _NC
    if _NC is None:
        _NC = build_bass()
    return _NC


def _shard_inputs(t1, t2, Wq, Wk, Wv, Wp, Wo, bo):
    t1 = np.asarray(t1, np.float32)
    t2 = np.asarray(t2, np.float32)
    x1ts = [np.ascontiguousarray(t1[b].T) for b in range(B)]
    x2ts = [np.ascontiguousarray(t2[b].T) for b in range(B)]
    Wq = np.asarray(Wq, np.float32)
    Wk = np.asarray(Wk, np.float32)
    Wv = np.asarray(Wv, np.float32)
    Wpn = -np.asarray(Wp, np.float32)
    Wo = np.asarray(Wo, np.float32)
    bo = np.asarray(bo, np.float32)
    bo_bcast = np.ascontiguousarray(np.broadcast_to(bo, (128, DIM)))
    zeros_b = np.zeros((128, DIM), np.float32)

    in_maps = []
    for c in range(NCORES):
        b = c // 4
        h0 = (2 * c) % 8
        hs = slice(64 * h0, 64 * h0 + 128)
        in_maps.append(
            {
                "x1t": x1ts[b],
                "x2t": x2ts[b],
                "wq": np.ascontiguousarray(Wq[:, hs]),
                "wk": np.ascontiguousarray(Wk[:, hs]),
                "wpn": np.ascontiguousarray(Wpn[:, hs]),
                "wv": np.ascontiguousarray(Wv[:, hs]),
                "wo": np.ascontiguousarray(Wo[hs, :]),
                "bo_b": bo_bcast if c % 4 == 0 else zeros_b,
            }
        )
    return in_maps


def kernel(t1, t2, Wq, Wk, Wv, Wp, Wo, bo, _trace=False):
    nc = _get_nc()
    in_maps = _shard_inputs(t1, t2, Wq, Wk, Wv, Wp, Wo, bo)
    res = run_bass_kernel_spmd(
        nc, in_maps, list(range(NCORES)), trace=_trace,
        tmpdir=os.environ.get("BASS_TMPDIR"),
    )
    parts = [r["out"] for r in res.results]
    out = np.zeros((B, N, DIM), np.float32)
    for b in range(B):
        out[b] = parts[4 * b] + parts[4 * b + 1] + parts[4 * b + 2] + parts[4 * b + 3]
    kernel.last_exec_time_ns = res.exec_time_ns
    kernel.last_results = res
    return out


if __name__ == "__main__":
    # quick CoreSim smoke test of one core
    from concourse.bass_interp import CoreSim
    import reference as ref

    inputs = {k: np.asarray(v) for k, v in ref.setup_inputs().items()}
    nc = build_bass()
    nc.finalize()
    in_maps = _shard_inputs(**inputs)
    core = int(os.environ.get("SMOKE_CORE", "0"))
    sim = CoreSim(nc)
    for k, v in in_maps[core].items():
        sim.tensor(k)[:] = v
    sim.simulate()
    got = np.array(sim.tensor("out"))
    # expected partial for this core
    t1, t2 = inputs["t1"], inputs["t2"]
    b = core // 4
    h0 = (2 * core) % 8
    acc = np.zeros((N, DIM), np.float32)
    for h in (h0, h0 + 1):
        q = (t1[b] @ inputs["Wq"][:, h * 64:(h + 1) * 64])
        kp = t1[b] @ inputs["Wk"][:, h * 64:(h + 1) * 64] - t2[b] @ inputs["Wp"][:, h * 64:(h + 1) * 64]
        v = t1[b] @ inputs["Wv"][:, h * 64:(h + 1) * 64]
        raw = q @ kp.T
        logits = SCALE * raw * raw
        s = np.exp(logits - logits.max(axis=1, keepdims=True))
        o = (s @ v) / s.sum(axis=1, keepdims=True)
        acc += o @ inputs["Wo"][h * 64:(h + 1) * 64, :]
    if core % 4 == 0:
        acc += inputs["bo"]
    err = np.abs(got - acc)
    denom = np.abs(acc).max()
    print("core", core, "absmax err:", err.max(), "rel:", err.max() / denom)

